# revision 1
# baseline (speedup 1.0000x reference)
"""Trainium2 Bass kernel for nn_AttentionModule (sparse axial-pooled attention).

Strategy: data-parallel over batch B=16 across 8 NeuronCores (2 images per
core), one SPMD program, no collectives.

Per image (H*W = 4096 pixels, C = 512):
  1. Load x pixel-tiles [128, 512], PE-transpose into xT [C(4x128), 4096].
  2. xmean via free-dim reduce of xT; qT = Wq^T @ xmean (+bq, /4096).
  3. K^T blocks = Wk^T @ xT (+bk), scores s^T = Qsel^T @ K^T per head-pair.
  4. E = exp(s/8) in pixel-major layout via small PE transposes (scores are
     O(0.1) so softmax needs no max subtraction).
  5. V = x @ Wv in natural layout; W = E * V elementwise (broadcast over v).
  6. Softmax numerators/denominators as masked-sum matmuls:
     Nv/Dv via stacked-identity mask (sum over h), Nh/Dh via a sliding
     block-ones mask (sum over w).  A_h = Nh/Dh + bv, A_v = Nv/Dv + bv.
  7. A^T = sum_n A_h (x) A_v via elementwise product + pair-sum matmul.
  8. out = A @ [Wo; bo] with a ones-row appended to A^T (bias for free).

All big matmuls run as float32r (full PE rate at N>=512), data stays fp32.
"""

import sys

sys.path.insert(0, "/opt/trn_rl_repo")

import numpy as np

import concourse.bass as bass
import concourse.tile as tile
from concourse import bacc, mybir
from concourse import bass_utils

F32 = mybir.dt.float32
F32R = mybir.dt.float32r
BF16 = mybir.dt.bfloat16

B, H, W, C = 16, 64, 64, 512
NHEAD, DK, DV, DO = 8, 64, 64, 512
NCORES = 8
BPC = B // NCORES          # images per core
NPIX = H * W               # 4096
NTILES = NPIX // 128       # 32 pixel tiles per image
NBLK = NPIX // 512         # 8 pixel blocks per image


def r(ap):
    """Bitcast an fp32 AP to float32r for full-rate PE matmul."""
    return ap.bitcast(F32R)


def _build_kernel():
    nc = bacc.Bacc("TRN2", target_bir_lowering=False, debug=False)

    dram = {}
    def din(name, shape):
        dram[name] = nc.dram_tensor(name, list(shape), F32, kind="ExternalInput").ap()
        return dram[name]

    x_d = din("x", (BPC, NPIX, C))
    wq_d = din("Wq", (C, NHEAD * DK))
    wk_d = din("Wk", (C, NHEAD * DK))
    wv_d = din("Wv", (C, NHEAD * DV))
    woe_d = din("Wo_ext", (DV + 1, DO))      # [Wo; bo]
    bq_d = din("bq", (NHEAD * DK,))
    bk_d = din("bk", (NHEAD * DK,))
    bv_d = din("bv", (NHEAD * DV,))
    id_d = din("ident", (128, 128))          # identity for PE transpose
    ii_d = din("ii64", (128, 64))            # two stacked 64-identities
    msk_d = din("masks", (NTILES, 128, 128)) # [Sel_h(t) | Sel_v] per tile

    out_d = nc.dram_tensor("out", [BPC, NPIX, DO], F32, kind="ExternalOutput").ap()

    with tile.TileContext(nc) as tc:
        _body(tc, x_d, wq_d, wk_d, wv_d, woe_d, bq_d, bk_d, bv_d,
              id_d, ii_d, msk_d, out_d)

    nc.compile()
    return nc


def _body(tc, x_d, wq_d, wk_d, wv_d, woe_d, bq_d, bk_d, bv_d,
          id_d, ii_d, msk_d, out_d):
    nc = tc.nc
    from contextlib import ExitStack
    ctx = ExitStack()

    const = ctx.enter_context(tc.tile_pool(name="const", bufs=1))
    xtp = ctx.enter_context(tc.tile_pool(name="xtp", bufs=1))
    xload = ctx.enter_context(tc.tile_pool(name="xload", bufs=4))
    epool = ctx.enter_context(tc.tile_pool(name="epool", bufs=NTILES + 2))
    wpool = ctx.enter_context(tc.tile_pool(name="wpool", bufs=3))
    small = ctx.enter_context(tc.tile_pool(name="small", bufs=2))
    att = ctx.enter_context(tc.tile_pool(name="att", bufs=2))
    ppool = ctx.enter_context(tc.tile_pool(name="ppool", bufs=3))
    atpool = ctx.enter_context(tc.tile_pool(name="atpool", bufs=2))

    # PSUM pools: 8 banks total.  big(4) + acc(2) + s(1) + e(1) = 8.
    ps_big = ctx.enter_context(tc.tile_pool(name="ps_big", bufs=4, space="PSUM"))
    ps_acc = ctx.enter_context(tc.tile_pool(name="ps_acc", bufs=1, space="PSUM"))
    ps_s = ctx.enter_context(tc.tile_pool(name="ps_s", bufs=1, space="PSUM"))
    ps_e = ctx.enter_context(tc.tile_pool(name="ps_e", bufs=1, space="PSUM"))

    # ---- constants into SBUF (once per core) ----
    id_sb = const.tile([128, 128], F32, tag="id")
    nc.sync.dma_start(id_sb[:], id_d)

    def load_r(shape, tag, src, dt=F32R):
        """DMA fp32 from DRAM, round-copy into an fp32r/bf16 tile."""
        stage = xload.tile(list(shape), F32, tag="xt")
        nc.sync.dma_start(stage[:], src)
        t = const.tile(list(shape), dt, tag=tag)
        nc.vector.tensor_copy(t[:], stage[:])
        return t

    ii_sb = load_r([128, 64], "ii", ii_d)
    woe_sb = load_r([DV + 1, DO], "woe", woe_d)
    msk_sb = const.tile([128, NTILES, 128], F32R, name="msk", tag="msk")
    for g in range(NTILES // 4):
        stage = xload.tile([128, 4, 128], F32, name="mstage", tag="xt")
        nc.sync.dma_start(stage[:], msk_d[4 * g:4 * g + 4].transpose([1, 0, 2]))
        nc.vector.tensor_copy(msk_sb[:, 4 * g:4 * g + 4, :], stage[:])

    wq_sb, wv_sb, bq_sb = [], [], []
    for j in range(4):
        wq_sb.append(load_r([128, 512], f"wq{j}", wq_d[j * 128:(j + 1) * 128, :]))
        wv_sb.append(load_r([128, 512], f"wv{j}", wv_d[j * 128:(j + 1) * 128, :]))
        t = const.tile([128, 1], F32, tag=f"bq{j}")
        nc.sync.dma_start(t[:], bq_d[j * 128:(j + 1) * 128].unsqueeze(1))
        bq_sb.append(t)

    # WkT[jo] = Wk^T chunk [(n,k) 128, C 512] via PE transposes.  bk is
    # dropped: it shifts every pixel's score by a per-head constant, which
    # both softmaxes cancel exactly.
    wkT = [const.tile([128, 512], F32R, name=f"wkT{jo}", tag=f"wkT{jo}")
           for jo in range(4)]
    for j in range(4):
        stage = xload.tile([128, 512], F32, tag="xt")
        nc.sync.dma_start(stage[:], wk_d[j * 128:(j + 1) * 128, :])
        pw = ps_big.tile([128, 512], F32, tag="big")
        for jo in range(4):
            nc.tensor.transpose(pw[:, jo * 128:(jo + 1) * 128],
                                stage[:, jo * 128:(jo + 1) * 128], id_sb[:])
        for jo in range(4):
            nc.vector.tensor_copy(wkT[jo][:, j * 128:(j + 1) * 128],
                                  pw[:, jo * 128:(jo + 1) * 128])

    bv0 = const.tile([1, 512], F32, tag="bv0")
    nc.sync.dma_start(bv0[:], bv_d.unsqueeze(0))
    bv_rep = const.tile([64, 512], F32, tag="bvrep")
    nc.gpsimd.partition_broadcast(bv_rep[:], bv0[:])

    z32 = const.tile([128, 8], F32, tag="z32")
    nc.vector.memset(z32[:], 0.0)
    ones32 = const.tile([1, 512], F32, tag="ones32")
    nc.vector.memset(ones32[:], 1.0)

    for b in range(BPC):
        # ---------- Phase A: load + transpose x ----------
        xT = xtp.tile([128, 4, NPIX], F32R, tag="xT")  # [C-part, chunk, pix]
        xs32 = small.tile([128, 4], F32, name="xs32", tag="xsum32")
        for t in range(NTILES):
            xt = xload.tile([128, 512], F32, tag="xt")
            nc.sync.dma_start(xt[:], x_d[b, t * 128:(t + 1) * 128, :])
            ps = ps_big.tile([128, 512], F32, tag="big")
            for j in range(4):
                nc.tensor.transpose(ps[:, j * 128:(j + 1) * 128],
                                    xt[:, j * 128:(j + 1) * 128], id_sb[:])
            dst = xT[:, :, t * 128:(t + 1) * 128]
            srcv = ps[:].rearrange("p (j f) -> p j f", j=4)
            if t % 2 == 0:
                nc.vector.tensor_copy(dst, srcv)   # rounds to fp32r
            else:
                nc.scalar.activation(dst, srcv,
                                     mybir.ActivationFunctionType.Copy)
            # per-tile pixel-sum partials, accumulated into xs32 [C, chunk]
            xpart = small.tile([128, 4], F32, name="xpart", tag="xpart")
            nc.vector.tensor_reduce(xpart[:], srcv,
                                    axis=mybir.AxisListType.X,
                                    op=mybir.AluOpType.add)
            if t == 0:
                nc.vector.tensor_copy(xs32[:], xpart[:])
            else:
                nc.vector.tensor_add(xs32[:], xs32[:], xpart[:])

        # ---------- Phase B: q ----------
        qt_sb = []
        for jo in range(4):
            qp = ps_e.tile([128, 1], F32, tag="eps")
            for j in range(4):
                nc.tensor.matmul(qp[:],
                                 wq_sb[j][:, jo * 128:(jo + 1) * 128].bitcast(F32),
                                 xs32[:, j:j + 1],
                                 start=(j == 0), stop=(j == 3))
            qt = small.tile([128, 1], F32, tag=f"qt{jo}")
            nc.scalar.activation(qt[:], qp[:],
                                 mybir.ActivationFunctionType.Identity,
                                 bias=bq_sb[jo][:], scale=1.0 / NPIX)
            qt_sb.append(qt)
        qsel = []
        for jo in range(4):
            qs = small.tile([128, 8], F32R, tag=f"qsel{jo}")
            nc.vector.tensor_copy(qs[:], z32[:])
            nc.vector.tensor_copy(qs[0:64, 2 * jo:2 * jo + 1], qt_sb[jo][0:64, :])
            nc.vector.tensor_copy(qs[64:128, 2 * jo + 1:2 * jo + 2],
                                  qt_sb[jo][64:128, :])
            qsel.append(qs)

        # ---------- Phase C: folded score weights + scores + E ----------
        # wqk[j][c, n] = sum_k Wk[c, (n,k)] q[n,k]  -> s^T = wqk^T @ xT
        wqk = []
        for j in range(4):
            wp = ps_e.tile([128, 8], F32, tag="eps")
            for jo in range(4):
                nc.tensor.matmul(wp[:], wkT[jo][:, j * 128:(j + 1) * 128],
                                 qsel[jo][:], start=(jo == 0), stop=(jo == 3))
            wq_t = small.tile([128, 8], F32R, tag=f"wqk{j}")
            nc.vector.tensor_copy(wq_t[:], wp[:])
            wqk.append(wq_t)
        e_tiles = []
        for kb in range(NBLK):
            sp = ps_s.tile([8, 512], F32, tag="sps")
            for j in range(4):
                nc.tensor.matmul(sp[:], wqk[j][:],
                                 xT[:, j, kb * 512:(kb + 1) * 512],
                                 start=(j == 0), stop=(j == 3))
            st = small.tile([8, 512], F32, tag="sT")
            nc.vector.tensor_copy(st[:], sp[:])
            ep = ps_e.tile([128, 32], F32, tag="eps")
            for tt in range(4):
                nc.tensor.transpose(ep[:, tt * 8:(tt + 1) * 8],
                                    st[:, tt * 128:(tt + 1) * 128],
                                    id_sb[0:8, 0:8])
            for tt in range(4):
                et = epool.tile([128, 8], F32R, tag="e")
                nc.scalar.activation(et[:], ep[:, tt * 8:(tt + 1) * 8],
                                     mybir.ActivationFunctionType.Exp,
                                     scale=1.0 / np.sqrt(DK))
                e_tiles.append(et)

        # ---------- Phase D: V + weighted sums ----------
        nhv = ps_acc.tile([128, 512], F32, tag="acc_nhv")
        dps = ps_acc.tile([128, 8], F32, tag="acc_d")
        for t in range(NTILES):
            vp = ps_big.tile([128, 512], F32, tag="big")
            for j in range(4):
                nc.tensor.matmul(vp[:], xT[:, j, t * 128:(t + 1) * 128],
                                 wv_sb[j][:], start=(j == 0), stop=(j == 3))
            et = e_tiles[t]
            wt = wpool.tile([128, 512], F32R, tag="w")
            nc.vector.tensor_tensor(
                wt[:].rearrange("p (n v) -> p n v", n=8),
                vp[:].rearrange("p (n v) -> p n v", n=8),
                et[:].bitcast(F32).unsqueeze(2).broadcast_to([128, 8, 64]),
                op=mybir.AluOpType.mult)
            nc.tensor.matmul(nhv[:], msk_sb[:, t, :], wt[:],
                             start=(t == 0), stop=(t == NTILES - 1))
            nc.tensor.matmul(dps[:], msk_sb[:, t, :], et[:],
                             start=(t == 0), stop=(t == NTILES - 1))

        # ---------- Phase E: normalize + transpose A_h, A_v ----------
        dr = small.tile([128, 8], F32, tag="dr")
        nc.vector.reciprocal(dr[:], dps[:])
        ah = att.tile([64, 512], F32, tag="ah")
        av = att.tile([64, 512], F32, tag="av")
        nc.vector.tensor_tensor(
            ah[:].rearrange("p (n v) -> p n v", n=8),
            nhv[0:64, :].rearrange("p (n v) -> p n v", n=8),
            dr[0:64, :].unsqueeze(2).broadcast_to([64, 8, 64]),
            op=mybir.AluOpType.mult)
        nc.vector.tensor_add(ah[:], ah[:], bv_rep[:])
        nc.vector.tensor_tensor(
            av[:].rearrange("p (n v) -> p n v", n=8),
            nhv[64:128, :].rearrange("p (n v) -> p n v", n=8),
            dr[64:128, :].unsqueeze(2).broadcast_to([64, 8, 64]),
            op=mybir.AluOpType.mult)
        nc.vector.tensor_add(av[:], av[:], bv_rep[:])

        ahT, avT = [], []
        for j in range(4):
            tp = ps_e.tile([128, 64], F32, tag="eps")
            nc.tensor.transpose(tp[:], ah[:, j * 128:(j + 1) * 128],
                                id_sb[0:64, 0:64])
            t_sb = att.tile([128, 64], F32, tag=f"ahT{j}")
            nc.vector.tensor_copy(t_sb[:], tp[:])
            ahT.append(t_sb)
            tp = ps_e.tile([128, 64], F32, tag="eps")
            nc.tensor.transpose(tp[:], av[:, j * 128:(j + 1) * 128],
                                id_sb[0:64, 0:64])
            t_sb = att.tile([128, 64], F32, tag=f"avT{j}")
            nc.vector.tensor_copy(t_sb[:], tp[:])
            avT.append(t_sb)

        # ---------- Phase F: combine + output projection ----------
        for g in range(NBLK):
            atp = ps_s.tile([64, 512], F32, tag="sps")
            for j in range(4):
                pt = ppool.tile([128, 512], F32R, tag="p")
                nc.vector.tensor_tensor(
                    pt[:].rearrange("p (h w) -> p h w", h=8),
                    ahT[j][:, g * 8:(g + 1) * 8].unsqueeze(2)
                        .broadcast_to([128, 8, 64]),
                    avT[j][:].unsqueeze(1).broadcast_to([128, 8, 64]),
                    op=mybir.AluOpType.mult)
                nc.tensor.matmul(atp[:], ii_sb[:], pt[:],
                                 start=(j == 0), stop=(j == 3))
            at_sb = atpool.tile([DV + 1, 512], F32R, tag="at")
            nc.scalar.activation(at_sb[0:64, :], atp[:],
                                 mybir.ActivationFunctionType.Copy)
            nc.scalar.activation(at_sb[64:65, :], ones32[:],
                                 mybir.ActivationFunctionType.Copy)
            for tt in range(4):
                op_ = ps_big.tile([128, 512], F32, tag="big")
                nc.tensor.matmul(op_[:], at_sb[:, tt * 128:(tt + 1) * 128],
                                 woe_sb[:], start=True, stop=True)
                ot = wpool.tile([128, 512], F32, tag="ow")
                if tt % 4 == 0:
                    nc.vector.tensor_copy(ot[:], op_[:])
                else:
                    nc.scalar.activation(ot[:], op_[:],
                                         mybir.ActivationFunctionType.Copy)
                pix0 = (g * 4 + tt) * 128
                nc.sync.dma_start(out_d[b, pix0:pix0 + 128, :], ot[:])

    ctx.close()


_NC_CACHE = None
PROFILE = False
PROFILE_DIR = None


def kernel(**inputs):
    global _NC_CACHE
    x = np.asarray(inputs["x"], dtype=np.float32)
    Wq = np.asarray(inputs["Wq"], dtype=np.float32)
    bq = np.asarray(inputs["bq"], dtype=np.float32)
    Wk = np.asarray(inputs["Wk"], dtype=np.float32)
    bk = np.asarray(inputs["bk"], dtype=np.float32)
    Wv = np.asarray(inputs["Wv"], dtype=np.float32)
    bv = np.asarray(inputs["bv"], dtype=np.float32)
    Wo = np.asarray(inputs["Wo"], dtype=np.float32)
    bo = np.asarray(inputs["bo"], dtype=np.float32)

    if _NC_CACHE is None:
        _NC_CACHE = _build_kernel()
    nc = _NC_CACHE

    woe = np.concatenate([Wo, bo[None, :]], axis=0)
    ident = np.eye(128, dtype=np.float32)
    ii64 = np.tile(np.eye(64, dtype=np.float32), (2, 1))
    masks = np.zeros((NTILES, 128, 128), dtype=np.float32)
    for t in range(NTILES):
        masks[t, 0:64, 2 * t] = 1.0        # Sel_h: h == 2t for first h-row
        masks[t, 64:128, 2 * t + 1] = 1.0  # Sel_h: h == 2t+1 for second
        masks[t, :, 64:128] = ii64         # Sel_v: w == p % 64
    shared = dict(Wq=Wq, Wk=Wk, Wv=Wv, Wo_ext=woe, bq=bq, bk=bk, bv=bv,
                  ident=ident, ii64=ii64, masks=masks)
    in_maps = []
    for c in range(NCORES):
        m = {"x": x[c * BPC:(c + 1) * BPC].reshape(BPC, NPIX, C).copy()}
        m.update(shared)
        in_maps.append(m)

    res = bass_utils.run_bass_kernel_spmd(nc, in_maps, core_ids=list(range(NCORES)),
                                          trace=PROFILE, tmpdir=PROFILE_DIR)
    if PROFILE:
        print("HW exec time:", res.exec_time_ns, "ns")
    outs = [res.results[c]["out"].reshape(BPC, H, W, DO) for c in range(NCORES)]
    return np.concatenate(outs, axis=0)


if __name__ == "__main__":
    rng = np.random.default_rng(0)
    ins = {
        "x": rng.standard_normal((B, H, W, C), dtype=np.float32),
        "Wq": rng.standard_normal((C, 512), dtype=np.float32) * 0.04,
        "bq": np.zeros(512, np.float32),
        "Wk": rng.standard_normal((C, 512), dtype=np.float32) * 0.04,
        "bk": np.zeros(512, np.float32),
        "Wv": rng.standard_normal((C, 512), dtype=np.float32) * 0.04,
        "bv": np.zeros(512, np.float32),
        "Wo": rng.standard_normal((64, 512), dtype=np.float32) * 0.1,
        "bo": np.zeros(512, np.float32),
    }
    out = kernel(**ins)
    print("kernel output", out.shape, out.dtype)

